# revision 7
# baseline (speedup 1.0000x reference)
"""CenterLoss kernel for Trainium2 (Bass/Tile), 8-core SPMD.

Math: the reference computes
    distmat = ||x||^2 + ||c||^2 - 2 x@c^T        [B, C]
    loss = sum(clip(distmat * onehot(labels), 1e-12, 1e12)) / B
Only the B label-gathered entries of distmat survive the mask; every other
element is clipped from 0 up to exactly 1e-12.  So
    loss = ( sum_i clip(||x_i - centers[labels_i]||^2, 1e-12, 1e12)
             + B*(C-1)*1e-12 ) / B
No BxC distmat is needed.

Sharding (num_classes axis, per the hint): centers are sharded across the 8
cores (6250 rows each); x and labels are replicated to every core.  Each
core is responsible for exactly the rows whose label lands in its shard.

Fast path (taken whenever every core owns <= 128 rows, which holds with
overwhelming probability for uniform labels; B=512 over 8 cores averages 64):
the host passes each core the *global row ids* it owns plus the *local
center ids* for those rows (pure index bookkeeping - no input data is
rearranged on the host).  The core then does two indirect-DMA gathers -
x rows from the replicated x, center rows from its shard - computes
||x_i - c||^2 per row on-device (subtract / square / row-reduce, split in
column halves so DVE and ACT pipeline), and returns the per-row squared
distances.  The host scatters per-core results back to row order, applies
the clip, adds the closed-form masked-zeros constant, and divides by B.

Fallback path (any core owns > 128 rows): every core processes all 512
rows against its shard with clamped local indices and a 0/1 ownership
mask; host sums partials across cores.  Identical math, ~2x slower.
"""

import numpy as np

B, D, C = 512, 1024, 50000
N_CORES = 8
S = C // N_CORES  # center rows per shard
P = 128  # SBUF partitions
NT = B // P  # row tiles of x (fallback path)
CLAMP_MIN = 1e-12
CLAMP_MAX = 1e12

_NC_CACHE = {}


def _new_nc():
    import concourse.bacc as bacc

    return bacc.Bacc(
        "TRN2",
        target_bir_lowering=False,
        debug=False,
        num_devices=N_CORES,
        num_swdge_queues=2,
    )


def _build_nc_fast():
    import concourse.bass as bass
    import concourse.mybir as mybir
    import concourse.tile as tile

    nc = _new_nc()
    x_d = nc.dram_tensor("x", [B, D], mybir.dt.float32, kind="ExternalInput")
    c_d = nc.dram_tensor("cshard", [S, D], mybir.dt.float32, kind="ExternalInput")
    r_d = nc.dram_tensor("rows", [P, 1], mybir.dt.int32, kind="ExternalInput")
    i_d = nc.dram_tensor("cidx", [P, 1], mybir.dt.int32, kind="ExternalInput")
    o_d = nc.dram_tensor("partial", [P, 1], mybir.dt.float32, kind="ExternalOutput")

    SPLIT = 2
    W = D // SPLIT
    with tile.TileContext(nc) as tc:
        with tc.tile_pool(name="sbuf", bufs=1) as pool:
            rows_sb = pool.tile([P, 1], mybir.dt.int32)
            nc.sync.dma_start(rows_sb[:], r_d[:])
            cidx_sb = pool.tile([P, 1], mybir.dt.int32)
            nc.sync.dma_start(cidx_sb[:], i_d[:])

            x_sb = pool.tile([P, D], mybir.dt.float32)
            nc.gpsimd.indirect_dma_start(
                out=x_sb[:],
                out_offset=None,
                in_=x_d[:, :],
                in_offset=bass.IndirectOffsetOnAxis(ap=rows_sb[:, :1], axis=0),
            )
            g_sb = pool.tile([P, D], mybir.dt.float32)
            nc.gpsimd.indirect_dma_start(
                out=g_sb[:],
                out_offset=None,
                in_=c_d[:, :],
                in_offset=bass.IndirectOffsetOnAxis(ap=cidx_sb[:, :1], axis=0),
            )

            diff = pool.tile([P, D], mybir.dt.float32)
            sq = pool.tile([P, D], mybir.dt.float32)
            rs = pool.tile([P, SPLIT], mybir.dt.float32)
            for h in range(SPLIT):
                sl = slice(h * W, (h + 1) * W)
                nc.vector.tensor_tensor(
                    out=diff[:, sl], in0=x_sb[:, sl], in1=g_sb[:, sl],
                    op=mybir.AluOpType.subtract,
                )
                nc.scalar.activation(
                    sq[:, sl], diff[:, sl], mybir.ActivationFunctionType.Square
                )
                nc.vector.reduce_sum(
                    rs[:, h : h + 1], sq[:, sl], axis=mybir.AxisListType.X
                )
            rst = pool.tile([P, 1], mybir.dt.float32)
            nc.vector.reduce_sum(rst[:], rs[:], axis=mybir.AxisListType.X)
            nc.sync.dma_start(o_d[:], rst[:])

    nc.compile()
    return nc


def _build_nc_fallback():
    import concourse.bass as bass
    import concourse.mybir as mybir
    import concourse.tile as tile

    nc = _new_nc()
    x_d = nc.dram_tensor("x", [B, D], mybir.dt.float32, kind="ExternalInput")
    c_d = nc.dram_tensor("cshard", [S, D], mybir.dt.float32, kind="ExternalInput")
    i_d = nc.dram_tensor("idx", [NT, P, 1], mybir.dt.int32, kind="ExternalInput")
    m_d = nc.dram_tensor("msk", [P, NT], mybir.dt.float32, kind="ExternalInput")
    o_d = nc.dram_tensor("partial", [P, NT], mybir.dt.float32, kind="ExternalOutput")

    with tile.TileContext(nc) as tc:
        with (
            tc.tile_pool(name="sbuf", bufs=2) as pool,
            tc.tile_pool(name="acc", bufs=1) as acc,
        ):
            msk_sb = acc.tile([P, NT], mybir.dt.float32)
            nc.sync.dma_start(msk_sb[:], m_d[:])
            rs_sb = acc.tile([P, NT], mybir.dt.float32)

            for t in range(NT):
                idx_sb = pool.tile([P, 1], mybir.dt.int32)
                nc.sync.dma_start(idx_sb[:], i_d[t])
                x_sb = pool.tile([P, D], mybir.dt.float32)
                nc.sync.dma_start(x_sb[:], x_d[t * P : (t + 1) * P, :])
                g_sb = pool.tile([P, D], mybir.dt.float32)
                nc.gpsimd.indirect_dma_start(
                    out=g_sb[:],
                    out_offset=None,
                    in_=c_d[:, :],
                    in_offset=bass.IndirectOffsetOnAxis(ap=idx_sb[:, :1], axis=0),
                )
                diff = pool.tile([P, D], mybir.dt.float32)
                nc.vector.tensor_tensor(
                    out=diff[:], in0=x_sb[:], in1=g_sb[:],
                    op=mybir.AluOpType.subtract,
                )
                sq = pool.tile([P, D], mybir.dt.float32)
                nc.scalar.activation(
                    sq[:], diff[:], mybir.ActivationFunctionType.Square
                )
                nc.vector.reduce_sum(
                    rs_sb[:, t : t + 1], sq[:], axis=mybir.AxisListType.X
                )

            rsm = acc.tile([P, NT], mybir.dt.float32)
            nc.vector.tensor_tensor(
                out=rsm[:], in0=rs_sb[:], in1=msk_sb[:], op=mybir.AluOpType.mult
            )
            nc.sync.dma_start(o_d[:], rsm[:])

    nc.compile()
    return nc


def _get_nc(which):
    if which not in _NC_CACHE:
        _NC_CACHE[which] = (
            _build_nc_fast() if which == "fast" else _build_nc_fallback()
        )
    return _NC_CACHE[which]


def _plan(labels_i):
    """Index bookkeeping for the fast path: which rows each core owns."""
    owner = labels_i // S
    rows_per_core = []
    for k in range(N_CORES):
        rows_k = np.nonzero(owner == k)[0].astype(np.int32)
        rows_per_core.append(rows_k)
    return rows_per_core


def _make_in_maps_fast(x, labels_i, centers, rows_per_core):
    in_maps = []
    for k in range(N_CORES):
        rows_k = rows_per_core[k]
        rows = np.zeros((P, 1), dtype=np.int32)
        cidx = np.zeros((P, 1), dtype=np.int32)
        n = len(rows_k)
        rows[:n, 0] = rows_k
        cidx[:n, 0] = (labels_i[rows_k] - k * S).astype(np.int32)
        in_maps.append(
            {
                "x": x,
                "cshard": centers[k * S : (k + 1) * S],
                "rows": rows,
                "cidx": cidx,
            }
        )
    return in_maps


def _make_in_maps_fallback(x, labels_i, centers):
    in_maps = []
    for k in range(N_CORES):
        lo = k * S
        local = np.clip(labels_i - lo, 0, S - 1).astype(np.int32)
        own = ((labels_i >= lo) & (labels_i < lo + S)).astype(np.float32)
        idx = local.reshape(NT, P, 1)
        msk = own.reshape(NT, P).T
        in_maps.append(
            {
                "x": x,
                "cshard": centers[lo : lo + S],
                "idx": np.ascontiguousarray(idx),
                "msk": np.ascontiguousarray(msk),
            }
        )
    return in_maps


def _loss_from_d(d):
    d = np.clip(d.astype(np.float64), CLAMP_MIN, CLAMP_MAX)
    loss = (d.sum() + B * (C - 1) * CLAMP_MIN) / B
    return np.array(loss, dtype=np.float32)


def _run_spmd(nc, in_maps, **kwargs):
    """run_bass_kernel_spmd with retries for transient device wedges."""
    import time as _time

    from concourse.bass_utils import run_bass_kernel_spmd

    last = None
    for attempt in range(3):
        try:
            return run_bass_kernel_spmd(
                nc, in_maps, core_ids=list(range(N_CORES)), **kwargs
            )
        except Exception as e:  # transient NRT/axon wedges heal on retry
            last = e
            _time.sleep(3.0 + 3.0 * attempt)
            try:  # poke the devices with a trivial op to help recovery
                import jax
                import jax.numpy as jnp

                jnp.add(jnp.ones((8, 8)), 1.0).block_until_ready()
            except Exception:
                pass
    raise last


def kernel(x, labels, centers):
    x = np.ascontiguousarray(np.asarray(x, dtype=np.float32))
    centers = np.ascontiguousarray(np.asarray(centers, dtype=np.float32))
    labels_i = np.asarray(labels).astype(np.int64).reshape(B)

    rows_per_core = _plan(labels_i)
    if max(len(r) for r in rows_per_core) <= P:
        nc = _get_nc("fast")
        in_maps = _make_in_maps_fast(x, labels_i, centers, rows_per_core)
        res = _run_spmd(nc, in_maps)
        d = np.zeros(B, dtype=np.float64)
        for k in range(N_CORES):
            rows_k = rows_per_core[k]
            out_k = res.results[k]["partial"].reshape(P)
            d[rows_k] = out_k[: len(rows_k)]
    else:
        nc = _get_nc("fallback")
        in_maps = _make_in_maps_fallback(x, labels_i, centers)
        res = _run_spmd(nc, in_maps)
        acc = np.zeros((P, NT), dtype=np.float64)
        for r in res.results:
            acc += r["partial"]
        d = acc.T.reshape(B)  # [p, t] -> row t*P+p
    return _loss_from_d(d)
